# revision 20
# baseline (speedup 1.0000x reference)
"""Trainium2 kernel for nn_ClusterForecasting: transformer-encoder forward on
host (tiny), all-pairs soft-DTW on 8 NeuronCores (dominant cost), final
softmax/top-k on host (tiny).

Device strategy (per sharding hint): shard the B^2=4096 pair axis across 8
cores -> 512 pairs/core. Each core runs the anti-diagonal wavefront DP with
pairs on the 128 SBUF partitions, 4 blocks of 128 pairs fused side-by-side in
the free dimension, cost-matrix diagonals computed on the fly from per-pair
series data via strided access patterns (series B pre-reversed on host so all
AP strides are +1).
"""

import numpy as np

B, T, F, DMODEL, DFF, H = 64, 128, 2, 64, 256, 4
GAMMA = 0.1
K_NEAREST = 5
BIG = 1e8
N_CORES = 8
PAIRS_PER_CORE = (B * B) // N_CORES        # 512
NBLK = PAIRS_PER_CORE // 128               # 4
WSTATE = 130                               # guard slot 0 + slots 1..129 (i=0..128)
WA = 132                                   # xa buffers (pad 1 front)
WB = 384                                   # reversed xb buffers


def _softmax(x, axis=-1):
    m = np.max(x, axis=axis, keepdims=True)
    e = np.exp(x - m)
    return e / np.sum(e, axis=axis, keepdims=True)


def _host_forward(x, w_emb, b_emb, wq, wk, wv, wo, w1, b1, w2, b2, w_down, b_down):
    x = np.asarray(x, np.float32)
    Bx, Tx, _ = x.shape
    h = x @ w_emb + b_emb                                     # [B,T,D]
    Dh = DMODEL // H
    q = (h @ wq).reshape(Bx, Tx, H, Dh)
    k = (h @ wk).reshape(Bx, Tx, H, Dh)
    v = (h @ wv).reshape(Bx, Tx, H, Dh)
    scores = np.einsum('bshd,bthd->bhst', q, k) / np.float32(np.sqrt(Dh))
    att = _softmax(scores.astype(np.float32), axis=-1).astype(np.float32)
    ctx = np.einsum('bhst,bthd->bshd', att, v).reshape(Bx, Tx, DMODEL)
    h = h + ctx @ wo
    h = h + np.maximum(h @ w1 + b1, np.float32(0.0)) @ w2 + b2
    x_rec = h @ w_down + b_down
    return np.ascontiguousarray(x_rec.astype(np.float32))


_NC_CACHE = {}


def _build_bass(cfg=None):
    """One SPMD Bass program: soft-DTW DP for 512 pairs (4 blocks x 128).

    Exact-window wavefront: at diagonal k only the valid slot range
    [lo..hi] is computed; the two window-edge slots of the new diagonal are
    set to BIG so later shifted reads see boundary values.
    """
    import concourse.tile as tile
    from concourse import bacc, mybir
    from concourse.alu_op_type import AluOpType as alu

    cfg = cfg or {}
    G = cfg.get("groups", 1)                 # staggered independent groups
    bufs = cfg.get("bufs", 3)
    fuse_exp = cfg.get("fuse_exp", False)    # overlapping-AP exp over prev1
    fold_c = cfg.get("fold_c", False)        # fold +c into cost diag on gps
    BPG = NBLK // G                          # blocks per group

    dt = mybir.dt.float32
    AF = mybir.ActivationFunctionType

    nc = bacc.Bacc("TRN2", target_bir_lowering=False, debug=False,
                   enable_asserts=False, num_devices=N_CORES)
    d_xa0 = nc.dram_tensor("xa0", [128, NBLK, WA], dt, kind="ExternalInput")
    d_xa1 = nc.dram_tensor("xa1", [128, NBLK, WA], dt, kind="ExternalInput")
    d_xb0 = nc.dram_tensor("xb0", [128, NBLK, WB], dt, kind="ExternalInput")
    d_xb1 = nc.dram_tensor("xb1", [128, NBLK, WB], dt, kind="ExternalInput")
    d_out = nc.dram_tensor("dtw_out", [128, NBLK, 1], dt, kind="ExternalOutput")

    with tile.TileContext(nc) as tc:
        with tc.tile_pool(name="data", bufs=1) as data_pool, \
             tc.tile_pool(name="state", bufs=1) as state_pool, \
             tc.tile_pool(name="tmp", bufs=bufs) as tmp:
            xa0 = data_pool.tile([128, NBLK, WA], dt, tag="xa0")
            xa1 = data_pool.tile([128, NBLK, WA], dt, tag="xa1")
            xb0 = data_pool.tile([128, NBLK, WB], dt, tag="xb0")
            xb1 = data_pool.tile([128, NBLK, WB], dt, tag="xb1")
            nc.sync.dma_start(xa0[:], d_xa0.ap())
            nc.sync.dma_start(xa1[:], d_xa1.ap())
            nc.sync.dma_start(xb0[:], d_xb0.ap())
            nc.sync.dma_start(xb1[:], d_xb1.ap())

            st = []   # per-group rotating state tiles
            cms = []  # per-group ring of per-diagonal min tiles [128,1]
            for g in range(G):
                rs = [state_pool.tile([128, BPG, WSTATE], dt,
                                      tag=f"g{g}r{i}", name=f"g{g}r{i}")
                      for i in range(3)]
                for r in rs:
                    nc.vector.memset(r[:], BIG)
                nc.vector.memset(rs[0][:, :, 1:2], 0.0)
                st.append(rs)
                ring = [state_pool.tile([128, 1], dt, tag=f"g{g}cm{i}",
                                        name=f"g{g}cm{i}") for i in range(4)]
                for cm in ring:
                    nc.vector.memset(cm[:], 0.0)
                cms.append(ring)

            def win(k):
                # computed slot range [clo..chi] = valid window +1 on each
                # side; edge cells self-saturate to +inf (never read by
                # valid cells), so no boundary writes are needed.
                clo = max(1, k - T)
                chi = min(T + 1, k + 1)
                return clo, chi, chi - clo + 1

            # stage-major emission across groups so independent group chains
            # interleave inside each engine's in-order queue
            def st_d(g, k):
                b0, b1e = g * BPG, (g + 1) * BPG
                clo, chi, L = win(k)
                oa = clo - 1
                ob = 255 - k + clo
                t1 = tmp.tile([128, BPG, L], dt, tag=f"t1g{g}", name="t1")
                t2 = tmp.tile([128, BPG, L], dt, tag=f"t2g{g}", name="t2")
                nc.gpsimd.tensor_sub(t1[:], xa0[:, b0:b1e, oa:oa + L],
                                     xb0[:, b0:b1e, ob:ob + L])
                nc.gpsimd.tensor_sub(t2[:], xa1[:, b0:b1e, oa:oa + L],
                                     xb1[:, b0:b1e, ob:ob + L])
                nc.gpsimd.tensor_mul(t1[:], t1[:], t1[:])
                nc.gpsimd.tensor_mul(t2[:], t2[:], t2[:])
                td = tmp.tile([128, BPG, L], dt, tag=f"tdg{g}", name="td")
                nc.gpsimd.tensor_add(td[:], t1[:], t2[:])
                return td

            bias_eng = cfg.get("bias_eng", "vector")

            def st_bias(g, k):
                bE = getattr(nc, bias_eng)
                cmb = tmp.tile([128, 1], dt, tag=f"cmbg{g}", name="cmb")
                b10 = tmp.tile([128, 1], dt, tag=f"b10g{g}", name="b10")
                bE.tensor_tensor(cmb[:], cms[g][(k - 2) % 4][:],
                                 cms[g][(k - 3) % 4][:], op=alu.min)
                bE.tensor_scalar_mul(b10[:], cmb[:], 1.0 / GAMMA)
                return cmb, b10

            def st_dc(g, k, td, cmb):
                # fold the bias offset into the cost diagonal (off-chain)
                if fold_c:
                    nc.gpsimd.tensor_scalar_add(td[:], td[:], cmb[:])

            def st_exp(g, k, b10):
                prev2, prev1, _ = st[g]
                clo, chi, L = win(k)
                ta = tmp.tile([128, 3, BPG, L], dt, tag=f"tag{g}", name="ta")
                if fuse_exp:
                    # one ACT op covers both prev1 reads (r1m, r1) via an
                    # overlapping access pattern: dim1 = [step 1, count 2]
                    ov = prev1[:, :, clo - 1:chi].unsqueeze(1)
                    ov.ap[1] = [1, 2]
                    nc.scalar.activation(ta[:, 0:2], ov, AF.Exp,
                                         scale=-1.0 / GAMMA, bias=b10[:])
                else:
                    nc.scalar.activation(ta[:, 0], prev1[:, :, clo - 1:chi],
                                         AF.Exp, scale=-1.0 / GAMMA,
                                         bias=b10[:])
                    nc.scalar.activation(ta[:, 1], prev1[:, :, clo:chi + 1],
                                         AF.Exp, scale=-1.0 / GAMMA,
                                         bias=b10[:])
                nc.scalar.activation(ta[:, 2], prev2[:, :, clo - 1:chi],
                                     AF.Exp, scale=-1.0 / GAMMA, bias=b10[:])
                return ta

            def st_se01(g, k, ta):
                nc.vector.tensor_add(ta[:, 0], ta[:, 0], ta[:, 1])

            def st_se(g, k, ta):
                nc.vector.tensor_add(ta[:, 0], ta[:, 0], ta[:, 2])

            def st_ln(g, k, ta):
                nc.scalar.activation(ta[:, 0], ta[:, 0], AF.Ln)

            def st_cur(g, k, ta, td, cmb):
                prev2, prev1, free = st[g]
                clo, chi, L = win(k)
                if fold_c:
                    # cur = -gamma*ln(se) + (d + c), one fused op
                    nc.vector.scalar_tensor_tensor(free[:, :, clo:chi + 1],
                                                   ta[:, 0], -GAMMA, td[:],
                                                   op0=alu.mult, op1=alu.add)
                else:
                    tq = tmp.tile([128, BPG, L], dt, tag=f"tqg{g}", name="tq")
                    nc.vector.scalar_tensor_tensor(tq[:], ta[:, 0], -GAMMA,
                                                   td[:], op0=alu.mult,
                                                   op1=alu.add)
                    nc.vector.tensor_scalar_add(free[:, :, clo:chi + 1],
                                                tq[:], cmb[:])
                nc.vector.tensor_reduce(cms[g][k % 4][:],
                                        free[:, :, clo:chi + 1],
                                        mybir.AxisListType.XY, alu.min)
                st[g] = [prev1, free, prev2]

            for k in range(2, 2 * T + 1):
                tds = [st_d(g, k) for g in range(G)]
                biases = [st_bias(g, k) for g in range(G)]
                for g in range(G):
                    st_dc(g, k, tds[g], biases[g][0])
                tas = [st_exp(g, k, biases[g][1]) for g in range(G)]
                for g in range(G):
                    st_se01(g, k, tas[g])
                for g in range(G):
                    st_se(g, k, tas[g])
                for g in range(G):
                    st_ln(g, k, tas[g])
                for g in range(G):
                    st_cur(g, k, tas[g], tds[g], biases[g][0])

            for g in range(G):
                t_out = tmp.tile([128, BPG, 1], dt, tag=f"t_outg{g}",
                                 name="t_out")
                nc.vector.tensor_copy(t_out[:], st[g][1][:, :, 129:130])
                nc.sync.dma_start(d_out.ap()[:, g * BPG:(g + 1) * BPG, :],
                                  t_out[:])
    nc.compile()
    return nc


def _get_nc():
    if "nc" not in _NC_CACHE:
        _NC_CACHE["nc"] = _build_bass()
    return _NC_CACHE["nc"]


def _prep_in_maps(x_rec):
    """Per-core input dicts: per-pair series data, pairs on partitions."""
    a_idx = np.arange(B * B) // B
    b_idx = np.arange(B * B) % B
    xa0 = np.zeros((B * B, WA), np.float32)
    xa1 = np.zeros((B * B, WA), np.float32)
    xa0[:, 1:T + 1] = x_rec[a_idx, :, 0]
    xa1[:, 1:T + 1] = x_rec[a_idx, :, 1]
    xb0 = np.zeros((B * B, WB), np.float32)
    xb1 = np.zeros((B * B, WB), np.float32)
    xb0[:, T:2 * T] = x_rec[b_idx, ::-1, 0]
    xb1[:, T:2 * T] = x_rec[b_idx, ::-1, 1]
    in_maps = []
    for m in range(N_CORES):
        sl = slice(m * PAIRS_PER_CORE, (m + 1) * PAIRS_PER_CORE)

        def shard(arr, W):
            # [512, W] -> [NBLK, 128, W] -> [128, NBLK, W]
            return np.ascontiguousarray(
                arr[sl].reshape(NBLK, 128, W).transpose(1, 0, 2))
        in_maps.append({
            "xa0": shard(xa0, WA), "xa1": shard(xa1, WA),
            "xb0": shard(xb0, WB), "xb1": shard(xb1, WB),
        })
    return in_maps


def _run_device_dtw(x_rec, trace=False):
    from concourse.bass_utils import run_bass_kernel_spmd
    nc = _get_nc()
    in_maps = _prep_in_maps(x_rec)
    res = run_bass_kernel_spmd(nc, in_maps, core_ids=list(range(N_CORES)),
                               trace=trace)
    dist = np.empty((B * B,), np.float32)
    for m in range(N_CORES):
        o = res.results[m]["dtw_out"]              # [128, NBLK, 1]
        dist[m * PAIRS_PER_CORE:(m + 1) * PAIRS_PER_CORE] = \
            o.reshape(128, NBLK).transpose(1, 0).reshape(-1)
    return dist.reshape(B, B), res


def kernel(**inputs):
    x_rec = _host_forward(
        inputs["x"], inputs["w_emb"], inputs["b_emb"], inputs["wq"],
        inputs["wk"], inputs["wv"], inputs["wo"], inputs["w1"], inputs["b1"],
        inputs["w2"], inputs["b2"], inputs["w_down"], inputs["b_down"])
    dtw_dist, _ = _run_device_dtw(x_rec)
    p = _softmax(-dtw_dist, axis=-1)
    k_nearest = np.argsort(-p, axis=-1, kind="stable")[:, :K_NEAREST].astype(np.int32)
    dist_knn = np.take_along_axis(dtw_dist, k_nearest, axis=-1)
    loss = np.mean(dist_knn, dtype=np.float32)
    return np.float32(loss), k_nearest, x_rec
